# revision 5
# baseline (speedup 1.0000x reference)
"""MoE layer (E=8 experts, top-2, swiGLU) on 8 TRN2 NeuronCores.

Strategy: expert parallelism. The router (x @ Wr -> top-2 -> softmax gates)
is tiny (<0.1% of FLOPs) and is computed on host to build the dispatch:
tokens are gathered per expert into a padded capacity-C batch, one expert
per core. Each core runs the expert MLP

    y = (silu(X @ W1a + b1a) * (X @ W1b + b1b)) @ W2 + b2

entirely on device in bf16 (err ~4e-3 << 2e-2 gate; bf16 matmul streams at
the same 1 col/cycle as f32r but halves DMA traffic and enables FWL fast
weight loads). The gate scale and the scatter-add combine happen on host.

Device kernel structure (per core, SPMD — identical program, per-core data):
  Single token block: W1 streamed exactly once; xt/hT/w2 SBUF-resident.
  - xt  [P, KO1, C]  tokens, transposed, natural layout           (resident)
  - w2  [P, KO2, D]  expert W2                                    (resident)
  - hT  [P, MP, C]   swiGLU output, transposed (H on partitions)  (resident)
  - W1 streamed from HBM in [P, 2, KO1, 128] column tiles, one per mp
  GEMM1: H1T[h, t] = sum_k W1[k, h] * X[t, k]  (stationary=W1,  moving=xt)
  GEMM2: YT[d, t]  = sum_h W2[h, d] * hT[h, t] (stationary=W2t, moving=hT)
  Both GEMMs stream the token dim as the moving operand in chunks of <=512
  (PSUM bank limit) — tokens never pad to 128-tiles, so total streamed
  columns hit the MAC-count minimum. Y leaves transposed [d, t]; the host
  combine undoes it.

  Prologue: the DMA queue delivers its first bytes only at ~9 us (engine
  preamble), so mp 0 uses a RAMP chunk table [16, 64, 128, 240, 512, ...]
  with the w1[0] tile split into quarter-loads interleaved between the
  first xt pieces on the sync queue. Real matmuls start as soon as the
  first ~70 KB land and pace themselves against DMA arrival — no blind
  warmup matmuls, no >3 us PE gap (which would re-throttle the HAM clock
  gate to 1.2 GHz).
  All DMA rides the single sync (HWDGE) queue, whose in-order service is
  exactly the critical path; w2 queues behind the last w1 tile and y
  writes stream out during GEMM2.
"""

import math

import numpy as np
import ml_dtypes

import concourse.bacc as bacc
import concourse.bass as bass  # noqa: F401
import concourse.mybir as mybir
import concourse.tile as tile
from concourse.bass_utils import run_bass_kernel_spmd

P = 128
NCORES = 8

f32 = mybir.dt.float32
bf16 = mybir.dt.bfloat16
SILU = mybir.ActivationFunctionType.Silu
ADD = mybir.AluOpType.add

NP_BF16 = ml_dtypes.bfloat16


def _ramp_chunks(C):
    """mp-0 chunk table: small chunks first so compute starts while the
    DMA queue is still ramping, then 512s; remainder last."""
    ramp = [16, 64, 128, 240]
    out = []
    c0 = 0
    for r in ramp:
        if c0 + r > C:
            break
        out.append((c0, r))
        c0 += r
    while c0 + 512 <= C:
        out.append((c0, 512))
        c0 += 512
    if c0 < C:
        out.append((c0, C - c0))
    return out


def _chunks(C):
    """Steady-state chunk table: 512s, remainder last."""
    out = []
    c0 = 0
    while c0 + 512 <= C:
        out.append((c0, 512))
        c0 += 512
    if c0 < C:
        out.append((c0, C - c0))
    return out


def build_moe_expert_nc(D, H, C, has_b1=False, has_b2=False):
    """Build the SPMD per-expert kernel. D % 128 == 0, H % 128 == 0,
    C % 16 == 0 required."""
    KO1 = D // P       # k tiles of GEMM1 (contraction over D)
    MP = H // P        # hidden tiles (per swiGLU half)
    KO2 = H // P       # k tiles of GEMM2 (contraction over H)
    DP = D // P        # GEMM2 output tiles over D
    ramp = _ramp_chunks(C)
    steady = _chunks(C)

    nc = bacc.Bacc(None)
    xt_d = nc.declare_dram_parameter("xt", [P, KO1, C], bf16, isOutput=False)
    w1_d = nc.declare_dram_parameter("w1", [MP, P, 2, KO1, P], bf16, isOutput=False)
    w2_d = nc.declare_dram_parameter("w2", [P, KO2, D], bf16, isOutput=False)
    if has_b1:
        b1_d = nc.declare_dram_parameter("b1", [P, 2, MP], f32, isOutput=False)
    if has_b2:
        b2_d = nc.declare_dram_parameter("b2", [P, DP], f32, isOutput=False)
    y_d = nc.declare_dram_parameter("y", [P, DP, C], f32, isOutput=True)

    with tile.TileContext(nc) as tc:
        with (
            tc.tile_pool(name="sb", bufs=1) as sb,
            tc.tile_pool(name="ps", bufs=1, space="PSUM") as ps,
        ):
            xt_sb = sb.tile([P, KO1, C], bf16)
            w2_sb = sb.tile([P, KO2, D], bf16)
            hT = sb.tile([P, MP, C], bf16)
            if has_b1:
                b1_sb = sb.tile([P, 2, MP], f32)
            if has_b2:
                b2_sb = sb.tile([P, DP], f32)

            # ---- prologue loads, interleaved on the in-order sync queue so
            # the first matmul group's operands arrive first ----
            w1t0 = sb.tile([P, 2, KO1, P], bf16, tag="w1t", bufs=3, name="w1t0")
            kq = KO1 // 2
            c0r, cwr = ramp[0]
            nc.sync.dma_start(w1t0[:, 0, :kq], w1_d[0, :, 0, :kq])
            nc.sync.dma_start(xt_sb[:, :, c0r : c0r + cwr], xt_d[:, :, c0r : c0r + cwr])
            nc.sync.dma_start(w1t0[:, 0, kq:], w1_d[0, :, 0, kq:])
            ramp_iter = iter(range(1, len(ramp)))
            for ci in (next(ramp_iter, None), next(ramp_iter, None)):
                if ci is not None:
                    c0, cw = ramp[ci]
                    nc.sync.dma_start(
                        xt_sb[:, :, c0 : c0 + cw], xt_d[:, :, c0 : c0 + cw]
                    )
            nc.sync.dma_start(w1t0[:, 1, :kq], w1_d[0, :, 1, :kq])
            for ci in (next(ramp_iter, None),):
                if ci is not None:
                    c0, cw = ramp[ci]
                    nc.sync.dma_start(
                        xt_sb[:, :, c0 : c0 + cw], xt_d[:, :, c0 : c0 + cw]
                    )
            nc.sync.dma_start(w1t0[:, 1, kq:], w1_d[0, :, 1, kq:])
            for ci in ramp_iter:
                c0, cw = ramp[ci]
                nc.sync.dma_start(xt_sb[:, :, c0 : c0 + cw], xt_d[:, :, c0 : c0 + cw])
            if has_b1:
                nc.sync.dma_start(b1_sb[:], b1_d[:])
            if has_b2:
                nc.sync.dma_start(b2_sb[:], b2_d[:])

            # ---- GEMM1 + swiGLU ----
            ic = 0
            for mp in range(MP):
                if mp == 0:
                    w1t = w1t0
                    chunks = ramp
                else:
                    w1t = sb.tile([P, 2, KO1, P], bf16, tag="w1t", bufs=3)
                    nc.sync.dma_start(w1t[:], w1_d[mp])
                    chunks = steady
                for c0, cw in chunks:
                    psa = ps.tile([P, 512], f32, tag=f"g1_{(2 * ic) % 6}")
                    psb = ps.tile([P, 512], f32, tag=f"g1_{(2 * ic + 1) % 6}")
                    ic += 1
                    for k in range(KO1):
                        nc.tensor.matmul(
                            psa[:, :cw],
                            lhsT=w1t[:, 0, k, :],
                            rhs=xt_sb[:, k, c0 : c0 + cw],
                            start=(k == 0),
                            stop=(k == KO1 - 1),
                        )
                    for k in range(KO1):
                        nc.tensor.matmul(
                            psb[:, :cw],
                            lhsT=w1t[:, 1, k, :],
                            rhs=xt_sb[:, k, c0 : c0 + cw],
                            start=(k == 0),
                            stop=(k == KO1 - 1),
                        )
                    # swiGLU: hT = silu(psa + b1a) * (psb + b1b)
                    sg = sb.tile([P, 512], f32, tag="sg", bufs=2)
                    if has_b1:
                        av = sb.tile([P, 512], f32, tag="av", bufs=2)
                        nc.vector.tensor_scalar_add(
                            av[:, :cw], psa[:, :cw], b1_sb[:, 0, mp : mp + 1]
                        )
                        nc.scalar.activation(sg[:, :cw], av[:, :cw], SILU)
                        bs = sb.tile([P, 512], f32, tag="bs", bufs=2)
                        nc.vector.tensor_scalar_add(
                            bs[:, :cw], psb[:, :cw], b1_sb[:, 1, mp : mp + 1]
                        )
                        nc.vector.tensor_mul(
                            hT[:, mp, c0 : c0 + cw], sg[:, :cw], bs[:, :cw]
                        )
                    else:
                        nc.scalar.activation(sg[:, :cw], psa[:, :cw], SILU)
                        nc.vector.tensor_mul(
                            hT[:, mp, c0 : c0 + cw], sg[:, :cw], psb[:, :cw]
                        )

            # w2 rides the sync queue behind the last w1 tile (arrives ~40 us
            # before GEMM2 needs it); y writes below queue after it.
            nc.sync.dma_start(w2_sb[:], w2_d[:])

            # ---- GEMM2: YT[d, t] — stationary w2 tile, moving hT ----
            iy = 0
            for dp in range(DP):
                for c0, cw in steady:
                    psy = ps.tile([P, 512], f32, tag=f"psy{iy % 2}")
                    iy += 1
                    for k in range(KO2):
                        nc.tensor.matmul(
                            psy[:, :cw],
                            lhsT=w2_sb[:, k, dp * P : (dp + 1) * P],
                            rhs=hT[:, k, c0 : c0 + cw],
                            start=(k == 0),
                            stop=(k == KO2 - 1),
                        )
                    ysb = sb.tile([P, 512], f32, tag="ysb", bufs=2)
                    if has_b2:
                        nc.vector.tensor_scalar_add(
                            ysb[:, :cw], psy[:, :cw], b2_sb[:, dp : dp + 1]
                        )
                    else:
                        nc.vector.tensor_copy(ysb[:, :cw], psy[:, :cw])
                    nc.sync.dma_start(y_d[:, dp, c0 : c0 + cw], ysb[:, :cw])
    # run_bass_via_pjrt (the axon execute path) takes a prebuilt module and
    # never finalizes it; Bacc defers register allocation to finalize().
    nc.finalize()
    return nc


def _route(x2, Wr):
    """Top-2 router, numpy fp32 (mirrors jax.lax.top_k + softmax)."""
    n = x2.shape[0]
    ar = np.arange(n)
    z = x2 @ Wr  # [N, E] fp32
    idx1 = z.argmax(axis=1)
    v1 = z[ar, idx1]
    z2 = z.copy()
    z2[ar, idx1] = -np.inf
    idx2 = z2.argmax(axis=1)
    v2 = z2[ar, idx2]
    m = np.maximum(v1, v2)
    e1 = np.exp(v1 - m)
    e2 = np.exp(v2 - m)
    s = e1 + e2
    return idx1, idx2, (e1 / s).astype(np.float32), (e2 / s).astype(np.float32)


def kernel(x, Wr, W1, b1, W2, b2):
    x = np.asarray(x, dtype=np.float32)
    Wr = np.asarray(Wr, dtype=np.float32)
    W1 = np.asarray(W1, dtype=np.float32)
    b1 = np.asarray(b1, dtype=np.float32)
    W2 = np.asarray(W2, dtype=np.float32)
    b2 = np.asarray(b2, dtype=np.float32)

    Bb, T, D = x.shape
    E, _, H2 = W1.shape
    H = H2 // 2
    N = Bb * T
    assert E == NCORES

    x2 = x.reshape(N, D)
    idx1, idx2, g1, g2 = _route(x2, Wr)

    tok = np.concatenate([np.arange(N), np.arange(N)])
    exp = np.concatenate([idx1, idx2])
    gat = np.concatenate([g1, g2])

    toks_e = [tok[exp == e] for e in range(E)]
    gats_e = [gat[exp == e] for e in range(E)]
    counts = np.array([len(t) for t in toks_e])
    C = max(512, int(math.ceil(counts.max() / 16) * 16))

    has_b1 = bool(np.any(b1))
    has_b2 = bool(np.any(b2))

    nc = build_moe_expert_nc(D, H, C, has_b1=has_b1, has_b2=has_b2)

    KO1 = D // P
    MP = H // P
    KO2 = H // P
    DP = D // P

    in_maps = []
    for e in range(E):
        ce = len(toks_e[e])
        xtf = np.zeros((D, C), dtype=NP_BF16)
        xtf[:, :ce] = x2[toks_e[e]].astype(NP_BF16).T
        xt_t = np.ascontiguousarray(
            xtf.reshape(KO1, P, C).transpose(1, 0, 2)
        )

        w1_t = np.ascontiguousarray(
            W1[e].astype(NP_BF16).reshape(KO1, P, 2, MP, P).transpose(3, 1, 2, 0, 4)
        )
        w2_t = np.ascontiguousarray(
            W2[e].astype(NP_BF16).reshape(KO2, P, D).transpose(1, 0, 2)
        )

        im = {"xt": xt_t, "w1": w1_t, "w2": w2_t}
        if has_b1:
            im["b1"] = np.ascontiguousarray(
                b1[e].reshape(2, MP, P).transpose(2, 0, 1)
            )
        if has_b2:
            im["b2"] = np.ascontiguousarray(
                b2[e].reshape(DP, P).T
            )
        in_maps.append(im)

    res = run_bass_kernel_spmd(nc, in_maps, list(range(NCORES)))

    out = np.zeros((N, D), dtype=np.float32)
    for e in range(E):
        ce = len(toks_e[e])
        # y is [P, DP, C] = YT[d % 128, d // 128, t]; undo the transpose and
        # apply the gates host-side
        yt = res.results[e]["y"]
        y2 = yt.transpose(2, 1, 0).reshape(-1, D)[:ce]
        out[toks_e[e]] += gats_e[e][:, None] * y2
    return out.reshape(Bb, T, D)
